# revision 36
# baseline (speedup 1.0000x reference)
"""GCN layer (GCNConv + relu + dense + relu) on 8 Trainium2 NeuronCores.

Strategy (v2 — PE segment-sum, no dma_scatter_add)
--------------------------------------------------
Math: out = relu(relu(GCNConv(x)) @ W_dense + b_dense) with
GCNConv(x)[v] = dinv[v] * sum_{e: src->v} dinv[src] * (x W_gcn)[src] + b_gcn
(self-loops included as ordinary edges; dinv = rsqrt(indegree incl. self).)

Device plan (2 SPMD launches over 8 cores, nodes sharded 12500/core):
  Launch A: per core, g = dinv_row * (x @ W_gcn) as bf16 rows padded to
            128 cols (256B — the dma_gather minimum granule).  Host
            pre-transposes x (tile-contiguous) so each tile is one matmul;
            4 tiles share one in/out DMA (launch A is dispatch-bound).
            Also computes dinv and ships it node-linear via DRAM so launch
            B's critical path has no DVE Newton preamble.
            Host concatenates shards -> table [100000, 128] bf16.
  Launch B: per core, edges sorted by (dst-tile-group, src-chunk, dst-tile)
            in 64-slot units (cuts padding; PE matmul partition bases must
            be 0/64).  Per gather instruction (<=1024 idxs, the Q7
            idx-scratch limit): dma_gather table rows -> SBUF; DVE builds a
            one-hot [128e x 128d] from dst offsets via is_equal vs a
            host-sent iota; PE matmul lhsT=msgs[:, 0:32] rhs=onehot
            accumulates feature-major segment sums in PSUM (fp32, exact).
            Accumulation is tile-major: one OPEN PSUM group per bank at a
            time (a start=True while another group is open in the same bank
            destroys it on HW), each accumulator owns a full bank.
            Instruction-trailing pad slots carry negative indices, which
            the gather ucode drops per core at runtime.  Epilogue is
            feature-major: dinv columns via DVE, biases per-partition via
            ACT, dense layer is one matmul per 4-tile group.  Output is
            written feature-major [32, 12544]; host transposes (free).

The scatter side needs no gpsimd descriptor generation at all; the wall is
the gather's Q7 descriptor generation (~3.3ns/idx), ~94% gpsimd occupancy.
"""

import sys

if "/opt/trn_rl_repo" not in sys.path:
    sys.path.insert(0, "/opt/trn_rl_repo")

from dataclasses import dataclass

import numpy as np

import concourse.bacc as bacc
import concourse.mybir as mybir
from concourse import tile
from concourse.bass_utils import run_bass_kernel_spmd


@dataclass(frozen=True)
class Cfg:
    n_cores: int = 8
    nloc: int = 12500
    ntiles: int = 98              # 128-row dst tiles per core (12544 padded)
    in_dim: int = 128
    net_dim: int = 32
    row: int = 128                # table row width (bf16) = 256B
    n_chunks: int = 4             # src chunks of 25000 (int16 gather idx)
    chunk: int = 25000
    tg: int = 4                   # dst tiles per PSUM/epilogue group
    gmax: int = 8                 # max 128-edge batches per dma_gather (1024,
                                  # the Q7 idx-scratch hard limit)

    @property
    def npad(self):
        return self.ntiles * 128  # 12544

    @property
    def n(self):
        return self.nloc * self.n_cores


FULL = Cfg()
assert FULL.n == 100000 and FULL.chunk * FULL.n_chunks == FULL.n


def _f32(x):
    return np.ascontiguousarray(x, dtype=np.float32)


def wrap16(a):
    """Index array [n] -> [128, n//16] int16 layout dma_gather expects."""
    assert a.size % 16 == 0
    w = a.reshape(-1, 16).T
    return np.ascontiguousarray(np.tile(w, (8, 1)), dtype=np.int16)


def _emit_dinv(nc, pool, deg_d, p, w, name):
    """deg (int32 [p, w]) -> dinv = 1/sqrt(deg) with a Newton step."""
    deg_t = pool.tile([p, w], mybir.dt.int32, name=f"{name}_i", tag="dinv_i")
    degf_t = pool.tile([p, w], mybir.dt.float32, name=f"{name}_f", tag="dinv_f")
    r_t = pool.tile([p, w], mybir.dt.float32, name=f"{name}_r", tag="dinv_r")
    s_t = pool.tile([p, w], mybir.dt.float32, name=f"{name}_s", tag="dinv_s")
    dinv_t = pool.tile([p, w], mybir.dt.float32, name=f"{name}_v", tag="dinv_v")
    nc.sync.dma_start(out=deg_t[:], in_=deg_d[:])
    nc.vector.tensor_copy(out=degf_t[:], in_=deg_t[:])
    nc.vector.reciprocal(out=r_t[:], in_=degf_t[:])
    # Newton: r <- r * (2 - d * r) computed as -(r * (d*r - 2))
    nc.vector.tensor_tensor(out=s_t[:], in0=degf_t[:], in1=r_t[:], op=mybir.AluOpType.mult)
    nc.vector.tensor_scalar_add(out=s_t[:], in0=s_t[:], scalar1=-2.0)
    nc.vector.tensor_tensor(out=s_t[:], in0=s_t[:], in1=r_t[:], op=mybir.AluOpType.mult)
    nc.vector.tensor_scalar_mul(out=s_t[:], in0=s_t[:], scalar1=-1.0)
    nc.scalar.sqrt(dinv_t[:], s_t[:])
    return dinv_t


# ---------------------------------------------------------------- layout


class Layout:
    """Static (core-independent) slot/instruction structure for launch B.

    Edge slots are allocated in 16-slot units (the dma_gather index
    granularity) so per-(tile, chunk) padding is 16, not 128.  A gather
    instruction covers <= 64 units (1024 idxs); its output is viewed as
    128-slot batches, and a (tile, chunk) segment maps to a static list of
    (instr, batch, lo, hi) partition-range pieces for the PE matmuls.
    """

    UNIT = 64  # slots per unit; pieces start at 0/64 (legal PE bases)

    def __init__(self, cfg: Cfg, nb16):
        self.cfg = cfg
        self.nb16 = nb16  # [ntiles, n_chunks] units per (t, k)
        U = self.UNIT
        T, K, TG = cfg.ntiles, cfg.n_chunks, cfg.tg
        UMAX = cfg.gmax * 128 // U  # units per instr
        self.tg_sizes = []
        self.tk_units = {}       # (t, k) -> (u0, u1) global unit span
        self.instrs = []         # (k, u0, n_units, g, batch_base)
        self.tg_instrs = []
        ntg = (T + TG - 1) // TG
        u = 0
        B = 0
        instr_of_unit = []
        for g in range(ntg):
            tiles = list(range(g * TG, min((g + 1) * TG, T)))
            self.tg_sizes.append(len(tiles))
            self.tg_instrs.append([])
            for k in range(K):
                run_u0 = u
                for t in tiles:
                    n = int(nb16[t, k])
                    self.tk_units[(t, k)] = (u, u + n)
                    u += n
                run_nu = u - run_u0
                ni = (run_nu + UMAX - 1) // UMAX
                off = run_u0
                for i in range(ni):
                    sz = (run_nu + ni - 1 - i) // ni  # even split
                    idx = len(self.instrs)
                    self.instrs.append((k, off, sz, g, B))
                    self.tg_instrs[g].append(idx)
                    instr_of_unit.extend([idx] * sz)
                    B += (sz * U + 127) // 128
                    off += sz
                assert off == u
        self.NU = u
        self.NB = B
        self.NI = len(self.instrs)
        self.ntg = ntg
        # pieces per (t, k): list of (instr, batch_in_instr, lo, hi)
        self.tk_pieces = {}
        for (t, k), (u0, u1) in self.tk_units.items():
            pieces = []
            for uu in range(u0, u1):
                i = instr_of_unit[uu]
                slot = (uu - self.instrs[i][1]) * U
                j, lo = slot // 128, slot % 128
                if pieces and pieces[-1][0] == i and pieces[-1][1] == j \
                        and pieces[-1][3] == lo:
                    pieces[-1] = (i, j, pieces[-1][2], lo + U)
                else:
                    pieces.append((i, j, lo, lo + U))
            self.tk_pieces[(t, k)] = pieces


def make_layout(cfg: Cfg, counts):
    """counts: [n_cores, ntiles, n_chunks] edge counts -> static Layout."""
    mx = counts.max(axis=0)
    u = Layout.UNIT
    nb16 = np.maximum((mx + u - 1) // u, 1).astype(np.int64)
    return Layout(cfg, nb16)


# ---------------------------------------------------------------- launch A


def build_launch_a(cfg: Cfg):
    nc = bacc.Bacc(
        "TRN2", target_bir_lowering=False, debug=False, num_devices=cfg.n_cores
    )
    T, K, F, R = cfg.ntiles, cfg.in_dim, cfg.net_dim, cfg.row
    xT_d = nc.dram_tensor("xT", [T, K, 128], mybir.dt.float32, kind="ExternalInput")
    w_d = nc.dram_tensor("w", [K, F], mybir.dt.float32, kind="ExternalInput")
    deg_d = nc.dram_tensor("deg", [128, T], mybir.dt.int32, kind="ExternalInput")
    g_d = nc.dram_tensor("g", [cfg.npad, R], mybir.dt.bfloat16, kind="ExternalOutput")
    dv_d = nc.dram_tensor("dinv1", [cfg.npad], mybir.dt.float32, kind="ExternalOutput")

    with tile.TileContext(nc) as tc:
        with (
            tc.tile_pool(name="const", bufs=1) as cpool,
            tc.tile_pool(name="xin", bufs=4) as xpool,
            tc.tile_pool(name="gout", bufs=4) as gpool,
            tc.tile_pool(name="ph", bufs=4, space="PSUM") as php,
        ):
            w_t = cpool.tile([K, F], mybir.dt.float32)
            nc.sync.dma_start(out=w_t[:], in_=w_d[:])
            dinv_t = _emit_dinv(nc, cpool, deg_d, 128, T, "dinv")
            # ship dinv (node-linear) to DRAM so launch B skips the Newton
            # chain (its DVE preamble serialized ~130us ahead of the one-hots)
            nc.sync.dma_start(
                out=dv_d.ap().rearrange("(t p) -> p t", p=128), in_=dinv_t[:]
            )

            # 4 tiles per group: one in-DMA, 4 matmuls (each a closed PSUM
            # group, safe to share banks), one out-DMA -> 4x fewer sync-engine
            # DMA dispatches (launch A is dispatch-bound, not bandwidth-bound).
            GA = 4
            for q in range((T + GA - 1) // GA):
                t0 = q * GA
                tn = min(GA, T - t0)
                xT_t = xpool.tile([128, GA, 128], mybir.dt.float32, tag="x")
                nc.sync.dma_start(
                    out=xT_t[:, :tn, :],
                    in_=xT_d[t0 : t0 + tn].rearrange("a f c -> f a c"),
                )
                g_t = gpool.tile([128, GA, R], mybir.dt.bfloat16, tag="g")
                for j in range(tn):
                    h_p = php.tile([128, F], mybir.dt.float32, tag="h")
                    nc.tensor.matmul(
                        h_p[:],
                        xT_t[:, j, :],
                        w_t[:],
                        start=True,
                        stop=True,
                    )
                    nc.scalar.activation(
                        g_t[:, j, 0:F],
                        h_p[:],
                        mybir.ActivationFunctionType.Copy,
                        scale=dinv_t[:, t0 + j : t0 + j + 1],
                    )
                nc.vector.memset(g_t[:, :tn, F:R], 0.0)
                nc.sync.dma_start(
                    out=g_d[t0 * 128 : (t0 + tn) * 128, :].rearrange(
                        "(a p) c -> p a c", p=128
                    ),
                    in_=g_t[:, :tn, :],
                )
    nc.compile()
    return nc


# ---------------------------------------------------------------- launch B


def build_launch_b(cfg: Cfg, lay: Layout):
    nc = bacc.Bacc(
        "TRN2",
        target_bir_lowering=False,
        debug=False,
        num_devices=cfg.n_cores,
        num_swdge_queues=4,
    )
    F, R, TG = cfg.net_dim, cfg.row, cfg.tg
    NB, NI = lay.NB, lay.NI
    GM = cfg.gmax

    g_d = nc.dram_tensor("g", [cfg.n, R], mybir.dt.bfloat16, kind="ExternalInput")
    src_d = nc.dram_tensor(
        "src_i", [NI, 128, GM * 8], mybir.dt.int16, kind="ExternalInput"
    )
    dstv_d = nc.dram_tensor("dstv", [128, NB], mybir.dt.bfloat16, kind="ExternalInput")
    iota_d = nc.dram_tensor("iota", [128, 128], mybir.dt.bfloat16, kind="ExternalInput")
    dv1_d = nc.dram_tensor("dinv1", [cfg.npad], mybir.dt.float32, kind="ExternalInput")
    bg_d = nc.dram_tensor("bg", [F, 1], mybir.dt.float32, kind="ExternalInput")
    wd_d = nc.dram_tensor("wd", [F, F], mybir.dt.float32, kind="ExternalInput")
    bd_d = nc.dram_tensor("bd", [F, 1], mybir.dt.float32, kind="ExternalInput")
    out_d = nc.dram_tensor("out", [F, cfg.npad], mybir.dt.float32, kind="ExternalOutput")

    with tile.TileContext(nc) as tc:
        with (
            tc.tile_pool(name="const", bufs=1) as cpool,
            tc.tile_pool(name="idx", bufs=20) as ipool,
            tc.tile_pool(name="msg", bufs=20) as mpool,
            tc.tile_pool(name="oh", bufs=20) as opool,
            tc.tile_pool(name="epi", bufs=2) as epool,
            tc.tile_pool(name="acc", bufs=4, space="PSUM") as apool,
            tc.tile_pool(name="h2", bufs=2, space="PSUM") as hpool,
        ):
            iota_t = cpool.tile([128, 128], mybir.dt.bfloat16)
            nc.sync.dma_start(out=iota_t[:], in_=iota_d[:])
            dstv_t = cpool.tile([128, NB], mybir.dt.bfloat16)
            nc.sync.dma_start(out=dstv_t[:], in_=dstv_d[:])
            bg_t = cpool.tile([F, 1], mybir.dt.float32)
            nc.sync.dma_start(out=bg_t[:], in_=bg_d[:])
            wd_t = cpool.tile([F, F], mybir.dt.float32)
            nc.sync.dma_start(out=wd_t[:], in_=wd_d[:])
            bd_t = cpool.tile([F, 1], mybir.dt.float32)
            nc.sync.dma_start(out=bd_t[:], in_=bd_d[:])
            # dinv feature-major [32, npad]: replicated from the launch-A-
            # computed node-linear dinv.  The DMAs are emitted inside the TG
            # loop (after TG0's gathers are queued on the sync engine) so the
            # first dma_gather isn't stuck behind this 1.6MB preamble; dinv is
            # only needed by the first epilogue, ~25us in.
            dinv_t = cpool.tile([F, cfg.npad], mybir.dt.float32, name="dinvfm")

            for g in range(lay.ntg):
                tn = lay.tg_sizes[g]
                W = tn * 128
                c0 = g * TG * 128
                # ---- gathers + one-hots for all of this group's instrs
                tiles_of = {}
                for i in lay.tg_instrs[g]:
                    k, u0, nu, _g, bb = lay.instrs[i]
                    nidx = nu * lay.UNIT
                    nbt = (nidx + 127) // 128
                    nw = nidx // 16
                    idx_t = ipool.tile([128, GM * 8], mybir.dt.int16, tag="idx")
                    # NOTE: idx loads must stay on the sync engine — issuing
                    # them from nc.scalar (ACT HWDGE) crashes launch B on HW.
                    nc.sync.dma_start(out=idx_t[:, :nw], in_=src_d[i, :, :nw])
                    msg_t = mpool.tile([128, GM, R], mybir.dt.bfloat16, tag="m")
                    nc.gpsimd.dma_gather(
                        msg_t[:, :nbt, :],
                        g_d[k * cfg.chunk : (k + 1) * cfg.chunk, :],
                        idx_t[:, :nw],
                        nidx,
                        nidx,
                        R,
                        queue_num=i % 4,
                    )
                    oh_t = opool.tile([128, GM, 128], mybir.dt.bfloat16, tag="oh")
                    nc.vector.tensor_tensor(
                        out=oh_t[:, :nbt, :],
                        in0=iota_t[:].unsqueeze(1).broadcast_to((128, nbt, 128)),
                        in1=dstv_t[:, bb : bb + nbt]
                        .unsqueeze(2)
                        .broadcast_to((128, nbt, 128)),
                        op=mybir.AluOpType.is_equal,
                    )
                    tiles_of[i] = (msg_t, oh_t)

                if g == 0:
                    for f in range(F):
                        nc.sync.dma_start(
                            out=dinv_t[f : f + 1, :], in_=dv1_d.ap().unsqueeze(0)
                        )

                # ---- tile-major accumulation: one open PSUM group at a time,
                # each accumulator owns a full bank (start=True resets the
                # bank, so groups must not interleave within one).
                h1_t = epool.tile([F, TG * 128], mybir.dt.float32, tag="h1")
                for tl in range(tn):
                    t = g * TG + tl
                    acc_t = apool.tile([128, 512], mybir.dt.float32, tag="acc")
                    pieces = []
                    for k in range(cfg.n_chunks):
                        pieces.extend(lay.tk_pieces[(t, k)])
                    for j, (i, b, lo, hi) in enumerate(pieces):
                        msg_t, oh_t = tiles_of[i]
                        nc.tensor.matmul(
                            acc_t[0:F, 0:128],
                            msg_t[lo:hi, b, 0:F],
                            oh_t[lo:hi, b, :],
                            start=(j == 0),
                            stop=(j == len(pieces) - 1),
                        )
                    nc.vector.tensor_tensor(
                        out=h1_t[:, tl * 128 : (tl + 1) * 128],
                        in0=acc_t[0:F, 0:128],
                        in1=dinv_t[:, (g * TG + tl) * 128 : (g * TG + tl + 1) * 128],
                        op=mybir.AluOpType.mult,
                    )

                # ---- epilogue for the group
                r1_t = epool.tile([F, TG * 128], mybir.dt.float32, tag="r1")
                nc.scalar.activation(
                    r1_t[:, :W], h1_t[:, :W],
                    mybir.ActivationFunctionType.Relu, bias=bg_t[:],
                )
                h2_p = hpool.tile([F, 512], mybir.dt.float32, tag="h2")
                nc.tensor.matmul(
                    h2_p[:, :W], wd_t[:], r1_t[:, :W], start=True, stop=True
                )
                o_t = epool.tile([F, TG * 128], mybir.dt.float32, tag="o")
                nc.scalar.activation(
                    o_t[:, :W], h2_p[:, :W],
                    mybir.ActivationFunctionType.Relu, bias=bd_t[:],
                )
                nc.sync.dma_start(out=out_d[:, c0 : c0 + W], in_=o_t[:, :W])
    nc.compile()
    return nc


# ---------------------------------------------------------------- host side


def host_prep(x, edge_index, W_gcn, b_gcn, W_dense, b_dense, cfg: Cfg):
    n, nloc = cfg.n, cfg.nloc
    row = np.asarray(edge_index[0]).astype(np.int64)
    col = np.asarray(edge_index[1]).astype(np.int64)
    deg = (np.bincount(col, minlength=n) + 1).astype(np.int32)  # + self-loop

    W_gcn = _f32(W_gcn)
    b_gcn = _f32(b_gcn).reshape(cfg.net_dim, 1)
    W_dense = _f32(W_dense)
    b_dense = _f32(b_dense).reshape(cfg.net_dim, 1)
    x = _f32(x)

    import ml_dtypes

    iota = np.tile(np.arange(128), (128, 1)).astype(ml_dtypes.bfloat16)

    # ---- per-core edge sets (dst-sharded) + self loops
    owner = col // nloc
    per_core = []
    counts = np.zeros((cfg.n_cores, cfg.ntiles, cfg.n_chunks), dtype=np.int64)
    for c in range(cfg.n_cores):
        m = owner == c
        srcs = row[m]
        dstl = col[m] - c * nloc
        loop = np.arange(nloc, dtype=np.int64)
        srcs = np.concatenate([srcs, loop + c * nloc])
        dstl = np.concatenate([dstl, loop])
        t = dstl >> 7
        k = srcs // cfg.chunk
        np.add.at(counts[c], (t, k), 1)
        per_core.append((srcs, dstl, t, k))

    lay = make_layout(cfg, counts)

    in_a, in_b = [], []
    for c in range(cfg.n_cores):
        srcs, dstl, t, k = per_core[c]
        # deg layouts
        dpad = np.ones(cfg.npad, dtype=np.int32)
        dpad[:nloc] = deg[c * nloc : (c + 1) * nloc]
        deg_a = np.ascontiguousarray(dpad.reshape(cfg.ntiles, 128).T)  # [128, T]

        xpad = np.zeros((cfg.npad, cfg.in_dim), dtype=np.float32)
        xpad[:nloc] = x[c * nloc : (c + 1) * nloc]
        xT3 = np.ascontiguousarray(
            xpad.reshape(cfg.ntiles, 128, cfg.in_dim).transpose(0, 2, 1)
        )
        in_a.append({"xT": xT3, "w": W_gcn, "deg": deg_a})

        # ---- slot assignment: unit stream in (TG, k, t) layout order
        U = lay.UNIT
        src_slots = np.zeros(lay.NU * U, dtype=np.int64)
        dst_slots = np.full(lay.NU * U, -1.0, dtype=np.float64)
        base = np.zeros((cfg.ntiles, cfg.n_chunks), dtype=np.int64)
        for (tt, kk), (u0, _u1) in lay.tk_units.items():
            base[tt, kk] = u0 * U
        order = np.lexsort((k, t))
        ts, ks = t[order], k[order]
        so = (srcs[order] - ks * cfg.chunk).astype(np.int64)
        do = (dstl[order] & 127).astype(np.int64)
        grp = ts * cfg.n_chunks + ks
        uq, starts_, cnts_ = np.unique(grp, return_index=True, return_counts=True)
        rank = np.arange(grp.size) - np.repeat(starts_, cnts_)
        slot = base[ts, ks] + rank
        src_slots[slot] = so
        dst_slots[slot] = do

        src_i = np.zeros((lay.NI, 128, cfg.gmax * 8), dtype=np.int16)
        dstv = np.full((128, lay.NB), -1.0, dtype=np.float64)
        for i, (kk, u0, nu, _g, bb) in enumerate(lay.instrs):
            # NOTE: do NOT mark trailing pads with negative indices — the
            # gather ucode truncates them at runtime, desyncing the DGE ring
            # against the decode-side reservation (hard device crash).
            seg = src_slots[u0 * U : (u0 + nu) * U]
            src_i[i, :, : nu * U // 16] = wrap16(seg)
            dseg = dst_slots[u0 * U : (u0 + nu) * U]
            nbt = (nu * U + 127) // 128
            pad = np.full(nbt * 128, -1.0)
            pad[: nu * U] = dseg
            dstv[:, bb : bb + nbt] = pad.reshape(nbt, 128).T
        dstv = np.ascontiguousarray(dstv).astype(ml_dtypes.bfloat16)

        in_b.append(
            {
                "src_i": src_i,
                "dstv": dstv,
                "iota": iota,
                "bg": b_gcn,
                "wd": W_dense,
                "bd": b_dense,
            }
        )
    return in_a, in_b, lay


def assemble_table(res_a, cfg: Cfg):
    return np.ascontiguousarray(
        np.concatenate([res_a[c]["g"][: cfg.nloc] for c in range(cfg.n_cores)], axis=0)
    )


def assemble_out(res_b, cfg: Cfg):
    return np.ascontiguousarray(
        np.concatenate(
            [res_b[c]["out"][:, : cfg.nloc].T for c in range(cfg.n_cores)], axis=0
        )
    ).astype(np.float32)


def _add_table(in_b, table, res_a, cfg: Cfg):
    for c, m in enumerate(in_b):
        m["g"] = table
        m["dinv1"] = np.ascontiguousarray(res_a[c]["dinv1"], dtype=np.float32)


def kernel(x, edge_index, W_gcn, b_gcn, W_dense, b_dense):
    cfg = FULL
    in_a, in_b, lay = host_prep(x, edge_index, W_gcn, b_gcn, W_dense, b_dense, cfg)
    nc_a = build_launch_a(cfg)
    nc_b = build_launch_b(cfg, lay)
    core_ids = list(range(cfg.n_cores))
    res_a = run_bass_kernel_spmd(nc_a, in_a, core_ids).results
    table = assemble_table(res_a, cfg)
    _add_table(in_b, table, res_a, cfg)
    res_b = run_bass_kernel_spmd(nc_b, in_b, core_ids).results
    return assemble_out(res_b, cfg)


# revision 38
# speedup vs baseline: 1.1396x; 1.1396x over previous
"""GCN layer (GCNConv + relu + dense + relu) on 8 Trainium2 NeuronCores.

Strategy (v2 — PE segment-sum, no dma_scatter_add)
--------------------------------------------------
Math: out = relu(relu(GCNConv(x)) @ W_dense + b_dense) with
GCNConv(x)[v] = dinv[v] * sum_{e: src->v} dinv[src] * (x W_gcn)[src] + b_gcn
(self-loops included as ordinary edges; dinv = rsqrt(indegree incl. self).)

Device plan (2 SPMD launches over 8 cores, nodes sharded 12500/core):
  Launch A: per core, g = dinv_row * (x @ W_gcn) as bf16 rows padded to
            128 cols (256B — the dma_gather minimum granule).  Host
            pre-transposes x (tile-contiguous) so each tile is one matmul;
            4 tiles share one in/out DMA (launch A is dispatch-bound).
            Also computes dinv and ships it node-linear via DRAM so launch
            B's critical path has no DVE Newton preamble.
            Host concatenates shards -> table [100000, 128] bf16.
  Launch B: per core, edges sorted by (dst-tile-group, src-chunk, dst-tile)
            in 64-slot units (cuts padding; PE matmul partition bases must
            be 0/64).  Per gather instruction (<=1024 idxs, the Q7
            idx-scratch limit): dma_gather table rows -> SBUF; DVE builds a
            one-hot [128e x 128d] from dst offsets via is_equal vs a
            host-sent iota; PE matmul lhsT=msgs[:, 0:32] rhs=onehot
            accumulates feature-major segment sums in PSUM (fp32, exact).
            Accumulation is tile-major: one OPEN PSUM group per bank at a
            time (a start=True while another group is open in the same bank
            destroys it on HW), each accumulator owns a full bank.
            Instruction-trailing pad slots carry negative indices, which
            the gather ucode drops per core at runtime.  Epilogue is
            feature-major: dinv columns via DVE, biases per-partition via
            ACT, dense layer is one matmul per 4-tile group.  Output is
            written feature-major [32, 12544]; host transposes (free).

The scatter side needs no gpsimd descriptor generation at all; the wall is
the gather's Q7 descriptor generation (~3.3ns/idx), ~94% gpsimd occupancy.
"""

import sys

if "/opt/trn_rl_repo" not in sys.path:
    sys.path.insert(0, "/opt/trn_rl_repo")

from dataclasses import dataclass

import numpy as np

import concourse.bacc as bacc
import concourse.mybir as mybir
from concourse import tile
from concourse.bass_utils import run_bass_kernel_spmd


@dataclass(frozen=True)
class Cfg:
    n_cores: int = 8
    nloc: int = 12500
    ntiles: int = 98              # 128-row dst tiles per core (12544 padded)
    in_dim: int = 128
    net_dim: int = 32
    row: int = 128                # table row width (bf16) = 256B
    n_chunks: int = 4             # src chunks of 25000 (int16 gather idx)
    chunk: int = 25000
    tg: int = 4                   # dst tiles per PSUM/epilogue group
    gmax: int = 8                 # max 128-edge batches per dma_gather (1024,
                                  # the Q7 idx-scratch hard limit)

    @property
    def npad(self):
        return self.ntiles * 128  # 12544

    @property
    def n(self):
        return self.nloc * self.n_cores


FULL = Cfg()
assert FULL.n == 100000 and FULL.chunk * FULL.n_chunks == FULL.n


def _f32(x):
    return np.ascontiguousarray(x, dtype=np.float32)


def wrap16(a):
    """Index array [n] -> [128, n//16] int16 layout dma_gather expects."""
    assert a.size % 16 == 0
    w = a.reshape(-1, 16).T
    return np.ascontiguousarray(np.tile(w, (8, 1)), dtype=np.int16)


def _emit_dinv(nc, pool, deg_d, p, w, name):
    """deg (int32 [p, w]) -> dinv = 1/sqrt(deg) with a Newton step."""
    deg_t = pool.tile([p, w], mybir.dt.int32, name=f"{name}_i", tag="dinv_i")
    degf_t = pool.tile([p, w], mybir.dt.float32, name=f"{name}_f", tag="dinv_f")
    r_t = pool.tile([p, w], mybir.dt.float32, name=f"{name}_r", tag="dinv_r")
    s_t = pool.tile([p, w], mybir.dt.float32, name=f"{name}_s", tag="dinv_s")
    dinv_t = pool.tile([p, w], mybir.dt.float32, name=f"{name}_v", tag="dinv_v")
    nc.sync.dma_start(out=deg_t[:], in_=deg_d[:])
    nc.vector.tensor_copy(out=degf_t[:], in_=deg_t[:])
    nc.vector.reciprocal(out=r_t[:], in_=degf_t[:])
    # Newton: r <- r * (2 - d * r) computed as -(r * (d*r - 2))
    nc.vector.tensor_tensor(out=s_t[:], in0=degf_t[:], in1=r_t[:], op=mybir.AluOpType.mult)
    nc.vector.tensor_scalar_add(out=s_t[:], in0=s_t[:], scalar1=-2.0)
    nc.vector.tensor_tensor(out=s_t[:], in0=s_t[:], in1=r_t[:], op=mybir.AluOpType.mult)
    nc.vector.tensor_scalar_mul(out=s_t[:], in0=s_t[:], scalar1=-1.0)
    nc.scalar.sqrt(dinv_t[:], s_t[:])
    return dinv_t


# ---------------------------------------------------------------- layout


class Layout:
    """Static (core-independent) slot/instruction structure for launch B.

    Edge slots are allocated in 16-slot units (the dma_gather index
    granularity) so per-(tile, chunk) padding is 16, not 128.  A gather
    instruction covers <= 64 units (1024 idxs); its output is viewed as
    128-slot batches, and a (tile, chunk) segment maps to a static list of
    (instr, batch, lo, hi) partition-range pieces for the PE matmuls.
    """

    UNIT = 64  # slots per unit; pieces start at 0/64 (legal PE bases)

    def __init__(self, cfg: Cfg, nb16):
        self.cfg = cfg
        self.nb16 = nb16  # [ntiles, n_chunks] units per (t, k)
        U = self.UNIT
        T, K, TG = cfg.ntiles, cfg.n_chunks, cfg.tg
        UMAX = cfg.gmax * 128 // U  # units per instr
        self.tg_sizes = []
        self.tk_units = {}       # (t, k) -> (u0, u1) global unit span
        self.instrs = []         # (k, u0, n_units, g, batch_base)
        self.tg_instrs = []
        ntg = (T + TG - 1) // TG
        u = 0
        B = 0
        instr_of_unit = []
        for g in range(ntg):
            tiles = list(range(g * TG, min((g + 1) * TG, T)))
            self.tg_sizes.append(len(tiles))
            self.tg_instrs.append([])
            for k in range(K):
                run_u0 = u
                for t in tiles:
                    n = int(nb16[t, k])
                    self.tk_units[(t, k)] = (u, u + n)
                    u += n
                run_nu = u - run_u0
                ni = (run_nu + UMAX - 1) // UMAX
                off = run_u0
                for i in range(ni):
                    sz = (run_nu + ni - 1 - i) // ni  # even split
                    idx = len(self.instrs)
                    self.instrs.append((k, off, sz, g, B))
                    self.tg_instrs[g].append(idx)
                    instr_of_unit.extend([idx] * sz)
                    B += (sz * U + 127) // 128
                    off += sz
                assert off == u
        self.NU = u
        self.NB = B
        self.NI = len(self.instrs)
        self.ntg = ntg
        # pieces per (t, k): list of (instr, batch_in_instr, lo, hi)
        self.tk_pieces = {}
        for (t, k), (u0, u1) in self.tk_units.items():
            pieces = []
            for uu in range(u0, u1):
                i = instr_of_unit[uu]
                slot = (uu - self.instrs[i][1]) * U
                j, lo = slot // 128, slot % 128
                if pieces and pieces[-1][0] == i and pieces[-1][1] == j \
                        and pieces[-1][3] == lo:
                    pieces[-1] = (i, j, pieces[-1][2], lo + U)
                else:
                    pieces.append((i, j, lo, lo + U))
            self.tk_pieces[(t, k)] = pieces


def make_layout(cfg: Cfg, counts):
    """counts: [n_cores, ntiles, n_chunks] edge counts -> static Layout."""
    mx = counts.max(axis=0)
    u = Layout.UNIT
    nb16 = np.maximum((mx + u - 1) // u, 1).astype(np.int64)
    return Layout(cfg, nb16)


# ---------------------------------------------------------------- launch A


def build_launch_a(cfg: Cfg):
    nc = bacc.Bacc(
        "TRN2", target_bir_lowering=False, debug=False, num_devices=cfg.n_cores
    )
    T, K, F, R = cfg.ntiles, cfg.in_dim, cfg.net_dim, cfg.row
    xT_d = nc.dram_tensor("xT", [T, K, 128], mybir.dt.float32, kind="ExternalInput")
    w_d = nc.dram_tensor("w", [K, F], mybir.dt.float32, kind="ExternalInput")
    deg_d = nc.dram_tensor("deg", [128, T], mybir.dt.int32, kind="ExternalInput")
    g_d = nc.dram_tensor("g", [cfg.npad, R], mybir.dt.bfloat16, kind="ExternalOutput")
    dv_d = nc.dram_tensor("dinv1", [cfg.npad], mybir.dt.float32, kind="ExternalOutput")

    with tile.TileContext(nc) as tc:
        with (
            tc.tile_pool(name="const", bufs=1) as cpool,
            tc.tile_pool(name="xin", bufs=4) as xpool,
            tc.tile_pool(name="gout", bufs=4) as gpool,
            tc.tile_pool(name="ph", bufs=4, space="PSUM") as php,
        ):
            w_t = cpool.tile([K, F], mybir.dt.float32)
            nc.sync.dma_start(out=w_t[:], in_=w_d[:])
            dinv_t = _emit_dinv(nc, cpool, deg_d, 128, T, "dinv")

            # 4 tiles per group: one in-DMA, 4 matmuls (each a closed PSUM
            # group, safe to share banks), one out-DMA -> 4x fewer sync-engine
            # DMA dispatches (launch A is dispatch-bound, not bandwidth-bound).
            GA = 4
            for q in range((T + GA - 1) // GA):
                t0 = q * GA
                tn = min(GA, T - t0)
                xT_t = xpool.tile([128, GA, 128], mybir.dt.float32, tag="x")
                nc.sync.dma_start(
                    out=xT_t[:, :tn, :],
                    in_=xT_d[t0 : t0 + tn].rearrange("a f c -> f a c"),
                )
                g_t = gpool.tile([128, GA, R], mybir.dt.bfloat16, tag="g")
                for j in range(tn):
                    h_p = php.tile([128, F], mybir.dt.float32, tag="h")
                    nc.tensor.matmul(
                        h_p[:],
                        xT_t[:, j, :],
                        w_t[:],
                        start=True,
                        stop=True,
                    )
                    nc.scalar.activation(
                        g_t[:, j, 0:F],
                        h_p[:],
                        mybir.ActivationFunctionType.Copy,
                        scale=dinv_t[:, t0 + j : t0 + j + 1],
                    )
                nc.vector.memset(g_t[:, :tn, F:R], 0.0)
                nc.sync.dma_start(
                    out=g_d[t0 * 128 : (t0 + tn) * 128, :].rearrange(
                        "(a p) c -> p a c", p=128
                    ),
                    in_=g_t[:, :tn, :],
                )

            # ship dinv (node-linear) to DRAM for launch B; emitted LAST so
            # its 12.5k element-descriptors don't delay the tile-load queue
            nc.sync.dma_start(
                out=dv_d.ap().rearrange("(t p) -> p t", p=128), in_=dinv_t[:]
            )
    nc.compile()
    return nc


# ---------------------------------------------------------------- launch B


def build_launch_b(cfg: Cfg, lay: Layout):
    nc = bacc.Bacc(
        "TRN2",
        target_bir_lowering=False,
        debug=False,
        num_devices=cfg.n_cores,
        num_swdge_queues=4,
    )
    F, R, TG = cfg.net_dim, cfg.row, cfg.tg
    NB, NI = lay.NB, lay.NI
    GM = cfg.gmax

    g_d = nc.dram_tensor("g", [cfg.n, R], mybir.dt.bfloat16, kind="ExternalInput")
    src_d = nc.dram_tensor(
        "src_i", [NI, 128, GM * 8], mybir.dt.int16, kind="ExternalInput"
    )
    dstv_d = nc.dram_tensor("dstv", [128, NB], mybir.dt.bfloat16, kind="ExternalInput")
    iota_d = nc.dram_tensor("iota", [128, 128], mybir.dt.bfloat16, kind="ExternalInput")
    dv1_d = nc.dram_tensor("dinv1", [cfg.npad], mybir.dt.float32, kind="ExternalInput")
    bg_d = nc.dram_tensor("bg", [F, 1], mybir.dt.float32, kind="ExternalInput")
    wd_d = nc.dram_tensor("wd", [F, F], mybir.dt.float32, kind="ExternalInput")
    bd_d = nc.dram_tensor("bd", [F, 1], mybir.dt.float32, kind="ExternalInput")
    out_d = nc.dram_tensor("out", [F, cfg.npad], mybir.dt.float32, kind="ExternalOutput")

    with tile.TileContext(nc) as tc:
        with (
            tc.tile_pool(name="const", bufs=1) as cpool,
            tc.tile_pool(name="idx", bufs=20) as ipool,
            tc.tile_pool(name="msg", bufs=20) as mpool,
            tc.tile_pool(name="oh", bufs=20) as opool,
            tc.tile_pool(name="epi", bufs=2) as epool,
            tc.tile_pool(name="acc", bufs=4, space="PSUM") as apool,
            tc.tile_pool(name="h2", bufs=2, space="PSUM") as hpool,
        ):
            iota_t = cpool.tile([128, 128], mybir.dt.bfloat16)
            nc.sync.dma_start(out=iota_t[:], in_=iota_d[:])
            dstv_t = cpool.tile([128, NB], mybir.dt.bfloat16)
            nc.sync.dma_start(out=dstv_t[:], in_=dstv_d[:])
            bg_t = cpool.tile([F, 1], mybir.dt.float32)
            nc.sync.dma_start(out=bg_t[:], in_=bg_d[:])
            wd_t = cpool.tile([F, F], mybir.dt.float32)
            nc.sync.dma_start(out=wd_t[:], in_=wd_d[:])
            bd_t = cpool.tile([F, 1], mybir.dt.float32)
            nc.sync.dma_start(out=bd_t[:], in_=bd_d[:])
            # dinv feature-major [32, npad]: replicated from the launch-A-
            # computed node-linear dinv.  The DMAs are emitted inside the TG
            # loop (after TG0's gathers are queued on the sync engine) so the
            # first dma_gather isn't stuck behind this 1.6MB preamble; dinv is
            # only needed by the first epilogue, ~25us in.
            dinv_t = cpool.tile([F, cfg.npad], mybir.dt.float32, name="dinvfm")

            for g in range(lay.ntg):
                tn = lay.tg_sizes[g]
                W = tn * 128
                c0 = g * TG * 128
                # ---- gathers + one-hots for all of this group's instrs
                tiles_of = {}
                for i in lay.tg_instrs[g]:
                    k, u0, nu, _g, bb = lay.instrs[i]
                    nidx = nu * lay.UNIT
                    nbt = (nidx + 127) // 128
                    nw = nidx // 16
                    idx_t = ipool.tile([128, GM * 8], mybir.dt.int16, tag="idx")
                    # NOTE: idx loads must stay on the sync engine — issuing
                    # them from nc.scalar (ACT HWDGE) crashes launch B on HW.
                    nc.sync.dma_start(out=idx_t[:, :nw], in_=src_d[i, :, :nw])
                    msg_t = mpool.tile([128, GM, R], mybir.dt.bfloat16, tag="m")
                    nc.gpsimd.dma_gather(
                        msg_t[:, :nbt, :],
                        g_d[k * cfg.chunk : (k + 1) * cfg.chunk, :],
                        idx_t[:, :nw],
                        nidx,
                        nidx,
                        R,
                        queue_num=i % 4,
                    )
                    oh_t = opool.tile([128, GM, 128], mybir.dt.bfloat16, tag="oh")
                    nc.vector.tensor_tensor(
                        out=oh_t[:, :nbt, :],
                        in0=iota_t[:].unsqueeze(1).broadcast_to((128, nbt, 128)),
                        in1=dstv_t[:, bb : bb + nbt]
                        .unsqueeze(2)
                        .broadcast_to((128, nbt, 128)),
                        op=mybir.AluOpType.is_equal,
                    )
                    tiles_of[i] = (msg_t, oh_t)

                if g == 0:
                    for f in range(F):
                        nc.sync.dma_start(
                            out=dinv_t[f : f + 1, :], in_=dv1_d.ap().unsqueeze(0)
                        )

                # ---- tile-major accumulation: one open PSUM group at a time,
                # each accumulator owns a full bank (start=True resets the
                # bank, so groups must not interleave within one).
                h1_t = epool.tile([F, TG * 128], mybir.dt.float32, tag="h1")
                for tl in range(tn):
                    t = g * TG + tl
                    acc_t = apool.tile([128, 512], mybir.dt.float32, tag="acc")
                    pieces = []
                    for k in range(cfg.n_chunks):
                        pieces.extend(lay.tk_pieces[(t, k)])
                    for j, (i, b, lo, hi) in enumerate(pieces):
                        msg_t, oh_t = tiles_of[i]
                        nc.tensor.matmul(
                            acc_t[0:F, 0:128],
                            msg_t[lo:hi, b, 0:F],
                            oh_t[lo:hi, b, :],
                            start=(j == 0),
                            stop=(j == len(pieces) - 1),
                        )
                    nc.vector.tensor_tensor(
                        out=h1_t[:, tl * 128 : (tl + 1) * 128],
                        in0=acc_t[0:F, 0:128],
                        in1=dinv_t[:, (g * TG + tl) * 128 : (g * TG + tl + 1) * 128],
                        op=mybir.AluOpType.mult,
                    )

                # ---- epilogue for the group
                r1_t = epool.tile([F, TG * 128], mybir.dt.float32, tag="r1")
                nc.scalar.activation(
                    r1_t[:, :W], h1_t[:, :W],
                    mybir.ActivationFunctionType.Relu, bias=bg_t[:],
                )
                h2_p = hpool.tile([F, 512], mybir.dt.float32, tag="h2")
                nc.tensor.matmul(
                    h2_p[:, :W], wd_t[:], r1_t[:, :W], start=True, stop=True
                )
                o_t = epool.tile([F, TG * 128], mybir.dt.float32, tag="o")
                nc.scalar.activation(
                    o_t[:, :W], h2_p[:, :W],
                    mybir.ActivationFunctionType.Relu, bias=bd_t[:],
                )
                nc.sync.dma_start(out=out_d[:, c0 : c0 + W], in_=o_t[:, :W])
    nc.compile()
    return nc


# ---------------------------------------------------------------- host side


def host_prep(x, edge_index, W_gcn, b_gcn, W_dense, b_dense, cfg: Cfg):
    n, nloc = cfg.n, cfg.nloc
    row = np.asarray(edge_index[0]).astype(np.int64)
    col = np.asarray(edge_index[1]).astype(np.int64)
    deg = (np.bincount(col, minlength=n) + 1).astype(np.int32)  # + self-loop

    W_gcn = _f32(W_gcn)
    b_gcn = _f32(b_gcn).reshape(cfg.net_dim, 1)
    W_dense = _f32(W_dense)
    b_dense = _f32(b_dense).reshape(cfg.net_dim, 1)
    x = _f32(x)

    import ml_dtypes

    iota = np.tile(np.arange(128), (128, 1)).astype(ml_dtypes.bfloat16)

    # ---- per-core edge sets (dst-sharded) + self loops
    owner = col // nloc
    per_core = []
    counts = np.zeros((cfg.n_cores, cfg.ntiles, cfg.n_chunks), dtype=np.int64)
    for c in range(cfg.n_cores):
        m = owner == c
        srcs = row[m]
        dstl = col[m] - c * nloc
        loop = np.arange(nloc, dtype=np.int64)
        srcs = np.concatenate([srcs, loop + c * nloc])
        dstl = np.concatenate([dstl, loop])
        t = dstl >> 7
        k = srcs // cfg.chunk
        np.add.at(counts[c], (t, k), 1)
        per_core.append((srcs, dstl, t, k))

    lay = make_layout(cfg, counts)

    in_a, in_b = [], []
    for c in range(cfg.n_cores):
        srcs, dstl, t, k = per_core[c]
        # deg layouts
        dpad = np.ones(cfg.npad, dtype=np.int32)
        dpad[:nloc] = deg[c * nloc : (c + 1) * nloc]
        deg_a = np.ascontiguousarray(dpad.reshape(cfg.ntiles, 128).T)  # [128, T]

        xpad = np.zeros((cfg.npad, cfg.in_dim), dtype=np.float32)
        xpad[:nloc] = x[c * nloc : (c + 1) * nloc]
        xT3 = np.ascontiguousarray(
            xpad.reshape(cfg.ntiles, 128, cfg.in_dim).transpose(0, 2, 1)
        )
        in_a.append({"xT": xT3, "w": W_gcn, "deg": deg_a})

        # ---- slot assignment: unit stream in (TG, k, t) layout order
        U = lay.UNIT
        src_slots = np.zeros(lay.NU * U, dtype=np.int64)
        dst_slots = np.full(lay.NU * U, -1.0, dtype=np.float64)
        base = np.zeros((cfg.ntiles, cfg.n_chunks), dtype=np.int64)
        for (tt, kk), (u0, _u1) in lay.tk_units.items():
            base[tt, kk] = u0 * U
        order = np.lexsort((k, t))
        ts, ks = t[order], k[order]
        so = (srcs[order] - ks * cfg.chunk).astype(np.int64)
        do = (dstl[order] & 127).astype(np.int64)
        grp = ts * cfg.n_chunks + ks
        uq, starts_, cnts_ = np.unique(grp, return_index=True, return_counts=True)
        rank = np.arange(grp.size) - np.repeat(starts_, cnts_)
        slot = base[ts, ks] + rank
        src_slots[slot] = so
        dst_slots[slot] = do

        src_i = np.zeros((lay.NI, 128, cfg.gmax * 8), dtype=np.int16)
        dstv = np.full((128, lay.NB), -1.0, dtype=np.float64)
        for i, (kk, u0, nu, _g, bb) in enumerate(lay.instrs):
            # NOTE: do NOT mark trailing pads with negative indices — the
            # gather ucode truncates them at runtime, desyncing the DGE ring
            # against the decode-side reservation (hard device crash).
            seg = src_slots[u0 * U : (u0 + nu) * U]
            src_i[i, :, : nu * U // 16] = wrap16(seg)
            dseg = dst_slots[u0 * U : (u0 + nu) * U]
            nbt = (nu * U + 127) // 128
            pad = np.full(nbt * 128, -1.0)
            pad[: nu * U] = dseg
            dstv[:, bb : bb + nbt] = pad.reshape(nbt, 128).T
        dstv = np.ascontiguousarray(dstv).astype(ml_dtypes.bfloat16)

        in_b.append(
            {
                "src_i": src_i,
                "dstv": dstv,
                "iota": iota,
                "bg": b_gcn,
                "wd": W_dense,
                "bd": b_dense,
            }
        )
    return in_a, in_b, lay


def assemble_table(res_a, cfg: Cfg):
    return np.ascontiguousarray(
        np.concatenate([res_a[c]["g"][: cfg.nloc] for c in range(cfg.n_cores)], axis=0)
    )


def assemble_out(res_b, cfg: Cfg):
    return np.ascontiguousarray(
        np.concatenate(
            [res_b[c]["out"][:, : cfg.nloc].T for c in range(cfg.n_cores)], axis=0
        )
    ).astype(np.float32)


def _add_table(in_b, table, res_a, cfg: Cfg):
    for c, m in enumerate(in_b):
        m["g"] = table
        m["dinv1"] = np.ascontiguousarray(res_a[c]["dinv1"], dtype=np.float32)


def kernel(x, edge_index, W_gcn, b_gcn, W_dense, b_dense):
    cfg = FULL
    in_a, in_b, lay = host_prep(x, edge_index, W_gcn, b_gcn, W_dense, b_dense, cfg)
    nc_a = build_launch_a(cfg)
    nc_b = build_launch_b(cfg, lay)
    core_ids = list(range(cfg.n_cores))
    res_a = run_bass_kernel_spmd(nc_a, in_a, core_ids).results
    table = assemble_table(res_a, cfg)
    _add_table(in_b, table, res_a, cfg)
    res_b = run_bass_kernel_spmd(nc_b, in_b, core_ids).results
    return assemble_out(res_b, cfg)
